# revision 41
# baseline (speedup 1.0000x reference)
"""GAT (2-layer, PyG-style) on 8 Trainium2 NeuronCores — v3.

Design vs v2:
  - 4 SWDGE queues round-robin: gather descriptor generation runs on all 4
    Q7 cpu pairs concurrently (queue q -> cpus 2q,2q+1) instead of one.
  - ONE row layout + ONE lo/hi boundary (slab-aligned, both halves <32768)
    shared by layer 1 and layer 2, so both layers use the SAME int16 index
    set, the same dloc, and the same per-block tile counts.
  - Layer-2 per-edge attention-dst logits (pad2) are computed at the end of
    each phase-B block (St is still resident) and kept in SBUF, so phase C
    needs no St/dlocT/one-hot rebuild at all.
  - Tables are stored p-major inside each 7-block slab so phase-A writes big
    contiguous runs (row r = slab*896 + p*7 + s).
  - table2 is one Shared buffer filled by a single AllGather.
  - leaky_relu on the scalar engine (Lrelu activation); softmax 'sume' adds
    read the pad PSUM directly (no bf16 staging copy).
  - phase C is pipelined: tdl + gathers issue CLA blocks ahead.
"""
import sys
sys.path.insert(0, "/opt/trn_rl_repo")

import numpy as np
import concourse.bass as bass
import concourse.bacc as bacc
import concourse.mybir as mybir
from concourse.tile import TileContext
from concourse.bass_utils import run_bass_kernel_spmd

F32 = mybir.dt.float32
BF16 = mybir.dt.bfloat16
F8 = mybir.dt.float8e4
I16 = mybir.dt.int16

P = 128
NCORES = 8
LEAKY = 0.2
EPS = 1e-16
NQ = 4                   # SWDGE queues (desc-gen cpu pairs)
GCHUNK = 7               # max tiles per dma_gather call (ring: idx/16+1 <= 64)


class Cfg:
    def __init__(self, N, E, IN_CH=256, HID=256, OUT_CH=64, H1=8):
        self.N, self.E = N, E
        self.IN_CH, self.HID, self.OUT_CH, self.H1 = IN_CH, HID, OUT_CH, H1
        self.C1 = HID // H1
        self.ND = N // NCORES                    # dst nodes per core
        self.NB = (self.ND + P - 1) // P         # dst blocks per core (49)
        self.NPAD = self.NB * P                  # padded shard rows
        self.R = NCORES * self.NPAD              # global table rows
        self.RB = NCORES * self.NB               # global table blocks (392)
        self.GA = 7                              # blocks per slab (49 = 7*7)
        assert self.NB % self.GA == 0
        self.NSLAB = self.NB // self.GA          # slabs per core (7)
        self.SLABR = self.GA * P                 # rows per (core, slab) = 896
        self.SLO = 4                             # slabs in the lo half
        # rows: [all cores' slabs 0..3 | all cores' slabs 4..6], core-major
        # within each half, p-major inside each slab. The lo half is complete
        # (and AllGather-able) once every core finishes block 27 of phase B,
        # so phase-C lo gathers overlap the tail of phase B.
        # lo/hi gather boundary: both halves < 32768 (int16)
        self.BND = self.SLO * NCORES * self.SLABR   # 28672
        assert self.BND < 32768 and (self.R - self.BND) < 32768
        self.TW1 = 384                           # table1 row bf16 (264 used)
        self.U1 = HID + H1                       # 264
        self.TW2 = 128                           # table2 row bf16 (65 used)
        self.U2 = OUT_CH + 1                     # 65 = [g 64 | as2]


CFG_FULL = Cfg(N=50000, E=800000)


# ---------------------------------------------------------------- host side
def _node_row(cfg, n):
    # [cores' slabs 0..3 | cores' slabs 4..6], p-major inside each slab:
    # lo: row = (c*4 + slab)*896 + p*7 + s;  hi: BND + (c*3 + slab-4)*896 + ...
    c = n // cfg.ND
    l = n - c * cfg.ND
    rb = l // P
    p = l % P
    slab = rb // cfg.GA
    s = rb % cfg.GA
    lo = (c * cfg.SLO + slab) * cfg.SLABR
    hi = cfg.BND + (c * (cfg.NSLAB - cfg.SLO) + slab - cfg.SLO) * cfg.SLABR
    return np.where(slab < cfg.SLO, lo, hi) + p * cfg.GA + s


def preprocess_graph(cfg, edge_index):
    """Per-core wrapped int16 gather indices (shared by both layers), dloc
    (edge-major) and dlocT (flat), plus per-block tile counts."""
    half = cfg.BND
    src = np.concatenate([edge_index[0], np.arange(cfg.N, dtype=np.int64)])
    dst = np.concatenate([edge_index[1], np.arange(cfg.N, dtype=np.int64)])
    r_src = _node_row(cfg, src)

    core = dst // cfg.ND
    dst_local = dst - core * cfg.ND
    blk = dst_local // P
    dloc = dst_local % P
    in_a = r_src < half

    NB = cfg.NB
    counts = np.zeros((NCORES, NB, 2), dtype=np.int64)
    np.add.at(counts, (core, blk, (~in_a).astype(np.int64)), 1)
    TA = np.maximum(1, (counts[:, :, 0].max(0) + P - 1) // P)
    TB = np.maximum(1, (counts[:, :, 1].max(0) + P - 1) // P)

    order = np.lexsort((in_a * -1, blk, core))  # by core, block, half (A first)
    rsrc_s, core_s, blk_s, dloc_s, ina_s = (
        r_src[order], core[order], blk[order], dloc[order], in_a[order])

    Tsum = int((TA + TB).sum())
    idx16 = []   # per core: [128, 8 * Tsum] int16
    dlocf = []   # per core: [128, Tsum] bf16-able float
    dloct = []   # per core: [1, Tsum*128] float
    for c in range(NCORES):
        iw = np.zeros((P, 8 * Tsum), dtype=np.int16)
        dw = np.full((P, Tsum), -1.0, dtype=np.float32)
        dt_ = np.full((1, Tsum * P), -1.0, dtype=np.float32)
        csel = core_s == c
        col0 = 0
        for b in range(NB):
            bsel = csel & (blk_s == b)
            for hh, T in ((0, int(TA[b])), (1, int(TB[b]))):
                hsel = bsel & (ina_s == (hh == 0))
                rr = rsrc_s[hsel] - (0 if hh == 0 else half)
                dd = dloc_s[hsel]
                S = T * P
                assert len(rr) <= S
                idx = np.zeros(S, dtype=np.int16)
                idx[: len(rr)] = rr.astype(np.int16)
                dl = np.full(S, -1.0, dtype=np.float32)
                dl[: len(dd)] = dd.astype(np.float32)
                w = idx.reshape(S // 16, 16).T          # [16, S/16] wrap
                iw[:, 8 * col0: 8 * col0 + S // 16] = np.tile(w, (8, 1))
                dw[:, col0: col0 + T] = dl.reshape(T, P).T
                dt_[0, col0 * P: col0 * P + S] = dl
                col0 += T
        assert col0 == Tsum
        idx16.append(iw)
        dlocf.append(dw)
        dloct.append(dt_)
    return idx16, dlocf, dloct, TA.astype(int).tolist(), TB.astype(int).tolist()


def make_weights(cfg, W1, att_src1, att_dst1, W2, att_src2, att_dst2):
    H1, C1 = cfg.H1, cfg.C1
    A1s = np.zeros((cfg.HID, H1), dtype=np.float64)
    A1s[np.arange(cfg.HID), np.arange(cfg.HID) // C1] = att_src1.ravel()
    A1d = np.zeros((cfg.HID, H1), dtype=np.float64)
    A1d[np.arange(cfg.HID), np.arange(cfg.HID) // C1] = att_dst1.ravel()
    # [h 256 | as 8 | ad 8] -> 272 cols
    W1f = np.concatenate([W1, W1 @ A1s, W1 @ A1d], axis=1).astype(np.float32)
    # [g 64 | as2 1 | ad2 1] -> 66 cols
    W2f = np.concatenate([W2, W2 @ att_src2.T, W2 @ att_dst2.T],
                         axis=1).astype(np.float32)
    return W1f, W2f


def bf16(a):
    import ml_dtypes
    return np.asarray(a, dtype=ml_dtypes.bfloat16)


# ---------------------------------------------------------------- device side
def _chunks(lo, hi):
    """Balanced <=GCHUNK-tile chunks of [lo, hi)."""
    T = hi - lo
    if T <= 0:
        return
    n = (T + GCHUNK - 1) // GCHUNK
    base, rem = divmod(T, n)
    c = lo
    for i in range(n):
        e = c + base + (1 if i < rem else 0)
        yield c, e
        c = e


def build_kernel(cfg, TA, TB, Tsum):
    nc = bacc.Bacc("TRN2", target_bir_lowering=False, debug=False,
                   num_devices=NCORES, num_swdge_queues=NQ)
    qctr = [0]

    def nextq():
        q = qctr[0] % NQ
        qctr[0] += 1
        return q

    IN, HID, OUT, H1, C1 = cfg.IN_CH, cfg.HID, cfg.OUT_CH, cfg.H1, cfg.C1
    U1, U2, TW1, TW2 = cfg.U1, cfg.U2, cfg.TW1, cfg.TW2
    NB, NPAD, R, RB = cfg.NB, cfg.NPAD, cfg.R, cfg.RB
    GA, BND = cfg.GA, cfg.BND
    SLAB = GA * P          # 896 rows per slab
    NG = RB // GA          # 56 phase-A groups
    KI = IN // P           # k-chunks for layer-1 dense
    KH = HID // P          # k-chunks for layer-2 dense
    W1W = U1 + H1          # 272
    LREL = mybir.ActivationFunctionType.Lrelu

    XTI = nc.declare_dram_parameter("XTI", [P, RB * KI * P], BF16, isOutput=False)
    XSI = nc.declare_dram_parameter("XSI", [P, NB * KI * P], BF16, isOutput=False)
    W1F = nc.declare_dram_parameter("W1F", [P, KI, W1W], BF16, isOutput=False)
    W2F = nc.declare_dram_parameter("W2F", [P, KH, U2 + 1], BF16, isOutput=False)
    TDL = nc.declare_dram_parameter("TDL", [P, 9 * Tsum], I16, isOutput=False)
    DLOCT = nc.declare_dram_parameter("DLOCT", [1, Tsum * P], BF16, isOutput=False)
    IOTA = nc.declare_dram_parameter("IOTA", [P, P], BF16, isOutput=False)
    IOTAC = nc.declare_dram_parameter("IOTAC", [P, 1], F32, isOutput=False)
    ONES1 = nc.declare_dram_parameter("ONES1", [1, P], BF16, isOutput=False)
    IDENT = nc.declare_dram_parameter("IDENT", [P, P], F32, isOutput=False)
    B1R = nc.declare_dram_parameter("B1R", [P, HID], F32, isOutput=False)
    B2R = nc.declare_dram_parameter("B2R", [P, OUT], F32, isOutput=False)
    OUTT = nc.declare_dram_parameter("OUTT", [cfg.ND, OUT], F32, isOutput=True)

    icols = []
    _ic = 0
    for b in range(NB):
        icols.append(_ic)
        _ic += TA[b] + TB[b]

    with TileContext(nc, num_cores=NCORES) as tc:
        with (
            tc.tile_pool(name="const", bufs=1) as cpool,
            tc.tile_pool(name="dram", bufs=1, space="DRAM") as dram,
        ):
            # resident constants
            iota_sb = cpool.tile([P, P], BF16)
            nc.sync.dma_start(out=iota_sb[:], in_=IOTA[:, :])
            iotac_sb = cpool.tile([P, 1], F32)
            nc.sync.dma_start(out=iotac_sb[:], in_=IOTAC[:, :])
            ones1_sb = cpool.tile([1, P], BF16)
            nc.sync.dma_start(out=ones1_sb[:], in_=ONES1[:, :])
            ident_sb = cpool.tile([P, P], F32)
            nc.sync.dma_start(out=ident_sb[:], in_=IDENT[:, :])
            b1_sb = cpool.tile([P, HID], F32)
            nc.sync.dma_start(out=b1_sb[:], in_=B1R[:, :])
            b2_sb = cpool.tile([P, OUT], F32)
            nc.sync.dma_start(out=b2_sb[:], in_=B2R[:, :])
            w1f_sb = cpool.tile([P, KI, W1W], BF16)
            for k in range(KI):
                nc.sync.dma_start(out=w1f_sb[:, k, :], in_=W1F[:, k, :])
            w2f_sb = cpool.tile([P, KH, U2 + 1], BF16)
            for k in range(KH):
                nc.sync.dma_start(out=w2f_sb[:, k, :], in_=W2F[:, k, :])
            # resident per-own-block attention-dst logits (layer 1) and
            # per-edge layer-2 dst logits (filled during phase B)
            ad1_sb = cpool.tile([P, NB, H1], F8)
            pad2_sb = cpool.tile([P, Tsum], BF16)

            table1 = dram.tile([R, TW1], BF16)
            shard2 = dram.tile([NPAD, TW2], BF16)
            table2_lo = dram.tile([BND, TW2], BF16, addr_space="Shared")
            table2_hi = dram.tile([R - BND, TW2], BF16, addr_space="Shared")
            t1_lo = table1[0:BND, :]
            t1_hi = table1[BND:R, :]
            t2_lo = table2_lo[:, :]
            t2_hi = table2_hi[:, :]
            SHLO = cfg.SLO * SLAB            # lo rows of the local shard2

            # ---------------- phase A: replicated layer-1 dense -> table1
            # phase-B pools open FIRST so B's gather destinations do not
            # alias phase-A tiles (aliasing would serialize B behind A)
            KLA = 3  # half-lo gather lookahead (blocks)
            KLB = 2  # half-hi gather + dlocT lookahead (blocks)
            CPRE = 8  # phase-C blocks whose lo gathers pre-issue in B's tail
            CLA = 6  # phase-C gather lookahead (blocks)
            import contextlib as _cl
            _stack = _cl.ExitStack()
            _stack_c = _cl.ExitStack()
            # pc pools first so the pb pools (closed earlier) stay LIFO
            p_geA = _stack_c.enter_context(tc.tile_pool(name="pc_geA", bufs=CPRE + 2))
            p_tdl2 = _stack_c.enter_context(
                tc.tile_pool(name="pc_tdl", bufs=CPRE + CLA + 2))
            p_heA = _stack.enter_context(tc.tile_pool(name="pb_heA", bufs=KLA + 2))
            p_heB = _stack.enter_context(tc.tile_pool(name="pb_heB", bufs=KLB + 2))
            p_tdl = _stack.enter_context(tc.tile_pool(name="pb_tdl", bufs=KLA + 2))
            sb_b = _stack.enter_context(tc.tile_pool(name="pb_sb", bufs=2))
            sm_b = _stack.enter_context(tc.tile_pool(name="pb_small", bufs=2))
            p_dlt = _stack.enter_context(tc.tile_pool(name="pb_dlt", bufs=KLB + 2))
            tdl2s = [None] * NB
            geAs = [None] * NB

            def issue_c_lo(cb):
                # phase-C tdl + lo-half gathers for block cb (table2_lo is
                # ready once the lo AllGather lands, at ~4/7 of phase B)
                Tc = TA[cb] + TB[cb]
                icc = icols[cb]
                tdl2 = p_tdl2.tile([P, 9 * Tc], I16, tag="tdl2")
                nc.sync.dma_start(out=tdl2[:],
                                  in_=TDL[:, 9 * icc: 9 * (icc + Tc)])
                tdl2s[cb] = tdl2
                geA = p_geA.tile([P, TA[cb], TW2], BF16, tag="geA")
                for c0, c1 in _chunks(0, TA[cb]):
                    nc.gpsimd.dma_gather(
                        geA[:, c0:c1, :], t2_lo, tdl2[:, 8 * c0:8 * c1],
                        num_idxs=(c1 - c0) * P, num_idxs_reg=(c1 - c0) * P,
                        elem_size=TW2, queue_num=nextq())
                geAs[cb] = geA
            with (
                tc.tile_pool(name="pa_sb", bufs=2) as sb,
                tc.tile_pool(name="pa_ps", bufs=4, space="PSUM") as ps,
            ):
                for g in range(NG):
                    xt = sb.tile([P, GA, KI, P], BF16, tag="xt")
                    nc.sync.dma_start(
                        out=xt[:],
                        in_=XTI[:, g * GA * KI * P:(g + 1) * GA * KI * P])
                    hrow = sb.tile([P, GA, TW1], BF16, tag="hrow")
                    nc.vector.memset(hrow[:, :, U1:TW1], 0.0)
                    for s in range(GA):
                        ph = ps.tile([P, U1], F32, tag="ph")
                        for k in range(KI):
                            nc.tensor.matmul(out=ph[:], lhsT=xt[:, s, k, :],
                                             rhs=w1f_sb[:, k, 0:U1],
                                             start=(k == 0), stop=(k == KI - 1))
                        if s % 2 == 0:
                            nc.scalar.copy(out=hrow[:, s, 0:U1], in_=ph[:])
                        else:
                            nc.vector.tensor_scalar(
                                out=hrow[:, s, 0:U1], in0=ph[:], scalar1=0.0,
                                scalar2=None, op0=mybir.AluOpType.add)
                    # p-major slab: row = g*896 + p*7 + s  -> contiguous
                    # 7*768B runs per partition
                    nc.scalar.dma_start(
                        out=table1[g * SLAB:(g + 1) * SLAB, :]
                        .rearrange("(p s) c -> p s c", s=GA),
                        in_=hrow[:])

                # phase A': own-shard attention-dst logits (tiny, resident)
                for g in range(NB // GA):
                    xs = sb.tile([P, GA, KI, P], BF16, tag="xs")
                    nc.sync.dma_start(
                        out=xs[:],
                        in_=XSI[:, g * GA * KI * P:(g + 1) * GA * KI * P])
                    for s in range(GA):
                        pa = ps.tile([P, H1], F32, tag="pa")
                        for k in range(KI):
                            nc.tensor.matmul(out=pa[:], lhsT=xs[:, s, k, :],
                                             rhs=w1f_sb[:, k, U1:W1W],
                                             start=(k == 0), stop=(k == KI - 1))
                        nc.scalar.copy(out=ad1_sb[:, g * GA + s, :], in_=pa[:])

            # ---------------- phase B: layer-1 edge aggregation + L2 dense
            with (
                tc.tile_pool(name="pb_ps", bufs=2, space="PSUM") as ps,
                tc.tile_pool(name="pb_ps1", bufs=2, space="PSUM") as ps1,
            ):
                sb = sb_b
                sm = sm_b
                tdls = [None] * NB
                heAs = [None] * NB
                heBs = [None] * NB
                dlts = [None] * NB
                for i in range(NB + KLA):
                    if i < NB:
                        # issue stage A: tdl load + half-lo gathers
                        b = i
                        T = TA[b] + TB[b]
                        ic = icols[b]
                        tdl = p_tdl.tile([P, 9 * T], I16, tag="tdl")
                        nc.sync.dma_start(out=tdl[:],
                                          in_=TDL[:, 9 * ic: 9 * (ic + T)])
                        tdls[b] = tdl
                        heA = p_heA.tile([P, TA[b], TW1], BF16, tag="heA")
                        for c0, c1 in _chunks(0, TA[b]):
                            nc.gpsimd.dma_gather(
                                heA[:, c0:c1, :], t1_lo,
                                tdl[:, 8 * c0:8 * c1],
                                num_idxs=(c1 - c0) * P,
                                num_idxs_reg=(c1 - c0) * P,
                                elem_size=TW1, queue_num=nextq())
                        heAs[b] = heA
                    ib = i - (KLA - KLB)
                    if 0 <= ib < NB:
                        # issue stage B: dlocT load + half-hi gathers, KLB
                        # blocks ahead of compute so the in-order vector
                        # stream never stalls waiting on them
                        T = TA[ib] + TB[ib]
                        TAb = TA[ib]
                        ic = icols[ib]
                        tdl = tdls[ib]
                        heB = p_heB.tile([P, TB[ib], TW1], BF16, tag="heB")
                        for c0, c1 in _chunks(0, TB[ib]):
                            nc.gpsimd.dma_gather(
                                heB[:, c0:c1, :], t1_hi,
                                tdl[:, 8 * (TAb + c0):8 * (TAb + c1)],
                                num_idxs=(c1 - c0) * P,
                                num_idxs_reg=(c1 - c0) * P,
                                elem_size=TW1, queue_num=nextq())
                        heBs[ib] = heB
                        dlocT = p_dlt.tile([1, T * P], BF16, tag="dlocT")
                        nc.scalar.dma_start(
                            out=dlocT[:], in_=DLOCT[:, ic * P: (ic + T) * P])
                        dlts[ib] = dlocT
                    if i < KLA:
                        continue
                    # compute stage (block b = i - KLA)
                    b = i - KLA
                    T = TA[b] + TB[b]
                    TAb = TA[b]
                    S128 = T * P
                    ic = icols[b]
                    tdl = tdls[b]
                    heA = heAs[b]
                    heB = heBs[b]
                    dlocT = dlts[b]
                    tdls[b] = heAs[b] = heBs[b] = dlts[b] = None
                    dloc = tdl[:, 8 * T:9 * T].bitcast(BF16)

                    # S[e, t, d] one-hot (edge-major) for aggregation
                    S = sb.tile([P, T, P], BF16, tag="S")
                    nc.vector.tensor_tensor(
                        out=S[:], in0=iota_sb[:].unsqueeze(1).to_broadcast([P, T, P]),
                        in1=dloc.unsqueeze(2).to_broadcast([P, T, P]),
                        op=mybir.AluOpType.is_equal)

                    # St[d, e] one-hot (dst-major) via rank-1 PE broadcast;
                    # 1024-wide is_equal halves the DVE per-op overhead
                    St = sb.tile([P, S128], F8, tag="St")
                    for c0 in range(0, S128, 1024):
                        c1 = min(c0 + 1024, S128)
                        stb = ps.tile([P, 1024], F32, tag="stb")
                        for d0 in range(c0, c1, 512):
                            d1 = min(d0 + 512, c1)
                            nc.tensor.matmul(out=stb[:, d0 - c0:d1 - c0],
                                             lhsT=ones1_sb[:],
                                             rhs=dlocT[:, d0:d1],
                                             start=True, stop=True)
                        nc.vector.tensor_scalar(
                            out=St[:, c0:c1], in0=stb[:, 0:c1 - c0],
                            scalar1=iotac_sb[:, 0:1], scalar2=None,
                            op0=mybir.AluOpType.is_equal)

                    # one PSUM bank holds all small per-block f32 scratch
                    assert T * H1 <= 192
                    blkp = ps1.tile([P, 512], F32, tag="blkp")
                    pad = blkp[:, 0:T * H1]
                    for t in range(T):
                        nc.tensor.matmul(
                            out=pad[:, t * H1:(t + 1) * H1],
                            lhsT=St[:, t * P:(t + 1) * P],
                            rhs=ad1_sb[:, b, :],
                            start=True, stop=True)

                    # exp(leaky(as + ad)) -> rhs[:, :, 0:H1]
                    sume = sm.tile([P, T * H1], BF16, tag="sume")
                    nc.vector.tensor_tensor(
                        out=sume[:, 0:TAb * H1].rearrange("p (t h) -> p t h", h=H1),
                        in0=heA[:, :, HID:U1],
                        in1=pad[:, 0:TAb * H1].rearrange("p (t h) -> p t h", h=H1),
                        op=mybir.AluOpType.add)
                    nc.vector.tensor_tensor(
                        out=sume[:, TAb * H1:].rearrange("p (t h) -> p t h", h=H1),
                        in0=heB[:, :, HID:U1],
                        in1=pad[:, TAb * H1:].rearrange("p (t h) -> p t h", h=H1),
                        op=mybir.AluOpType.add)
                    lk = sm.tile([P, T * H1], BF16, tag="lk")
                    nc.vector.scalar_tensor_tensor(
                        out=lk[:], in0=sume[:], scalar=LEAKY, in1=sume[:],
                        op0=mybir.AluOpType.mult, op1=mybir.AluOpType.max)
                    rhs = sb.tile([P, T, H1 + HID], BF16, tag="rhs")
                    nc.scalar.activation(
                        out=rhs[:, :, 0:H1],
                        in_=lk[:].rearrange("p (t h) -> p t h", h=H1),
                        func=mybir.ActivationFunctionType.Exp)
                    # Mw = h * ex (broadcast over the 32 chans of each head)
                    nc.vector.tensor_tensor(
                        out=rhs[:, 0:TAb, H1:].rearrange("p t (h c) -> p t h c", h=H1),
                        in0=heA[:, :, 0:HID].rearrange("p t (h c) -> p t h c", h=H1),
                        in1=rhs[:, 0:TAb, 0:H1].unsqueeze(3)
                        .to_broadcast([P, TAb, H1, C1]),
                        op=mybir.AluOpType.mult)
                    nc.vector.tensor_tensor(
                        out=rhs[:, TAb:T, H1:].rearrange("p t (h c) -> p t h c", h=H1),
                        in0=heB[:, :, 0:HID].rearrange("p t (h c) -> p t h c", h=H1),
                        in1=rhs[:, TAb:T, 0:H1].unsqueeze(3)
                        .to_broadcast([P, T - TAb, H1, C1]),
                        op=mybir.AluOpType.mult)

                    pm = ps.tile([P, H1 + HID], F32, tag="pm")
                    for t in range(T):
                        nc.tensor.matmul(out=pm[:], lhsT=S[:, t, :], rhs=rhs[:, t, :],
                                         start=(t == 0), stop=(t == T - 1))

                    # normalize + bias + ELU -> h2 block (f32)
                    srec = sm.tile([P, H1], F32, tag="srec")
                    nc.vector.tensor_scalar(
                        out=srec[:], in0=pm[:, 0:H1], scalar1=EPS, scalar2=None,
                        op0=mybir.AluOpType.add)
                    nc.vector.reciprocal(out=srec[:], in_=srec[:])
                    t2 = sm.tile([P, HID], F32, tag="t2")
                    nc.vector.tensor_tensor(
                        out=t2[:].rearrange("p (h c) -> p h c", h=H1),
                        in0=pm[:, H1:].rearrange("p (h c) -> p h c", h=H1),
                        in1=srec[:].unsqueeze(2).to_broadcast([P, H1, C1]),
                        op=mybir.AluOpType.mult)
                    nc.vector.tensor_tensor(out=t2[:], in0=t2[:], in1=b1_sb[:],
                                            op=mybir.AluOpType.add)
                    # ELU via scalar engine: min(t2,0) = -relu(-t2)
                    uu = sm.tile([P, HID], F32, tag="uu")
                    nc.scalar.activation(out=uu[:], in_=t2[:], scale=-1.0,
                                         func=mybir.ActivationFunctionType.Relu)
                    qq = sm.tile([P, HID], F32, tag="qq")
                    nc.scalar.activation(out=qq[:], in_=uu[:], scale=-1.0,
                                         func=mybir.ActivationFunctionType.Exp)
                    pp = sm.tile([P, HID], F32, tag="pp")
                    nc.scalar.activation(out=pp[:], in_=t2[:],
                                         func=mybir.ActivationFunctionType.Relu)
                    h2 = sm.tile([P, HID], F32, tag="h2")
                    nc.vector.scalar_tensor_tensor(
                        out=h2[:], in0=qq[:], scalar=-1.0, in1=pp[:],
                        op0=mybir.AluOpType.add, op1=mybir.AluOpType.add)

                    # layer-2 dense for this block: g_ext = h2 @ W2F
                    h2T = sm.tile([P, KH, P], BF16, tag="h2T")
                    for k in range(KH):
                        ptr2 = blkp[:, 192:320]
                        nc.tensor.transpose(out=ptr2[:], in_=h2[:, k * P:(k + 1) * P],
                                            identity=ident_sb[:])
                        nc.scalar.copy(out=h2T[:, k, :], in_=ptr2[:])
                    pg = blkp[:, 320:320 + U2 + 1]
                    for k in range(KH):
                        nc.tensor.matmul(out=pg[:], lhsT=h2T[:, k, :],
                                         rhs=w2f_sb[:, k, :],
                                         start=(k == 0), stop=(k == KH - 1))
                    gr = sm.tile([P, U2], BF16, tag="gr")
                    nc.scalar.copy(out=gr[:], in_=pg[:, 0:U2])
                    # p-major slab row: (b//7)*896 + p*7 + b%7
                    nc.scalar.dma_start(
                        out=shard2[(b // GA) * SLAB:(b // GA + 1) * SLAB, :]
                        .rearrange("(p s) c -> p s c", s=GA)[:, b % GA, 0:U2],
                        in_=gr[:])

                    # cache layer-2 per-edge dst logits: pad2 = St_t^T @ ad2
                    ad2c = sm.tile([P, 1], F8, tag="ad2c")
                    nc.scalar.copy(out=ad2c[:], in_=pg[:, U2:U2 + 1])
                    pad2p = blkp[:, 392:392 + T]
                    for t in range(T):
                        nc.tensor.matmul(
                            out=pad2p[:, t:t + 1],
                            lhsT=St[:, t * P:(t + 1) * P],
                            rhs=ad2c[:],
                            start=True, stop=True)
                    nc.scalar.copy(out=pad2_sb[:, ic:ic + T], in_=pad2p[:])

                    # AllGather the lo half (slabs 0..3) as soon as every core
                    # has written it (block 27), the rest at the end
                    if b == cfg.SLO * GA - 1:
                        nc.gpsimd.collective_compute(
                            "AllGather", mybir.AluOpType.bypass,
                            replica_groups=[list(range(NCORES))],
                            ins=[shard2[0:SHLO, :].opt()],
                            outs=[table2_lo[:, :].opt()])
                    elif b == NB - 1:
                        nc.gpsimd.collective_compute(
                            "AllGather", mybir.AluOpType.bypass,
                            replica_groups=[list(range(NCORES))],
                            ins=[shard2[SHLO:NPAD, :].opt()],
                            outs=[table2_hi[:, :].opt()])

                    # pre-issue phase-C lo gathers for the first CPRE blocks
                    # into B's tail, filling the idle SWDGE queue pairs
                    if b >= NB - CPRE:
                        issue_c_lo(b - (NB - CPRE))
            _stack.close()

            # ---------------- phase C: layer-2 edge aggregation (pipelined;
            # blocks 0..CPRE-1 had tdl+lo gathers pre-issued during phase B)
            with (
                tc.tile_pool(name="pc_geB", bufs=CLA + 2) as p_geB,
                tc.tile_pool(name="pc_sb", bufs=2) as sb,
                tc.tile_pool(name="pc_small", bufs=3) as sm,
                tc.tile_pool(name="pc_ps", bufs=2, space="PSUM") as ps,
            ):
                geBs = [None] * NB
                for i in range(NB + CLA):
                    if i < NB:
                        b = i
                        T = TA[b] + TB[b]
                        TAb = TA[b]
                        if b >= CPRE:
                            issue_c_lo(b)
                        tdl = tdl2s[b]
                        geB = p_geB.tile([P, TB[b], TW2], BF16, tag="geB")
                        for c0, c1 in _chunks(0, TB[b]):
                            nc.gpsimd.dma_gather(
                                geB[:, c0:c1, :], t2_hi,
                                tdl[:, 8 * (TAb + c0):8 * (TAb + c1)],
                                num_idxs=(c1 - c0) * P,
                                num_idxs_reg=(c1 - c0) * P,
                                elem_size=TW2, queue_num=nextq())
                        geBs[b] = geB
                    if i < CLA:
                        continue
                    b = i - CLA
                    T = TA[b] + TB[b]
                    TAb = TA[b]
                    ic = icols[b]
                    tdl = tdl2s[b]
                    geA = geAs[b]
                    geB = geBs[b]
                    tdl2s[b] = geAs[b] = geBs[b] = None
                    dloc = tdl[:, 8 * T:9 * T].bitcast(BF16)

                    S = sb.tile([P, T, P], BF16, tag="S")
                    nc.vector.tensor_tensor(
                        out=S[:], in0=iota_sb[:].unsqueeze(1).to_broadcast([P, T, P]),
                        in1=dloc.unsqueeze(2).to_broadcast([P, T, P]),
                        op=mybir.AluOpType.is_equal)

                    sum2 = sm.tile([P, T], BF16, tag="sum2")
                    nc.vector.tensor_tensor(
                        out=sum2[:, 0:TAb], in0=geA[:, :, OUT:U2].squeeze(2),
                        in1=pad2_sb[:, ic:ic + TAb], op=mybir.AluOpType.add)
                    nc.vector.tensor_tensor(
                        out=sum2[:, TAb:T], in0=geB[:, :, OUT:U2].squeeze(2),
                        in1=pad2_sb[:, ic + TAb:ic + T], op=mybir.AluOpType.add)
                    lk2 = sm.tile([P, T], BF16, tag="lk2")
                    nc.vector.scalar_tensor_tensor(
                        out=lk2[:], in0=sum2[:], scalar=LEAKY, in1=sum2[:],
                        op0=mybir.AluOpType.mult, op1=mybir.AluOpType.max)
                    rhs2 = sb.tile([P, T, 1 + OUT], BF16, tag="rhs2")
                    nc.scalar.activation(out=rhs2[:, :, 0:1],
                                         in_=lk2[:].unsqueeze(2),
                                         func=mybir.ActivationFunctionType.Exp)
                    nc.vector.tensor_tensor(
                        out=rhs2[:, 0:TAb, 1:],
                        in0=geA[:, :, 0:OUT],
                        in1=rhs2[:, 0:TAb, 0:1].to_broadcast([P, TAb, OUT]),
                        op=mybir.AluOpType.mult)
                    nc.vector.tensor_tensor(
                        out=rhs2[:, TAb:T, 1:],
                        in0=geB[:, :, 0:OUT],
                        in1=rhs2[:, TAb:T, 0:1].to_broadcast([P, T - TAb, OUT]),
                        op=mybir.AluOpType.mult)

                    pm2 = ps.tile([P, 1 + OUT], F32, tag="pm")
                    for t in range(T):
                        nc.tensor.matmul(out=pm2[:], lhsT=S[:, t, :],
                                         rhs=rhs2[:, t, :],
                                         start=(t == 0), stop=(t == T - 1))

                    rec2 = sm.tile([P, 1], F32, tag="rec2")
                    nc.vector.tensor_scalar(
                        out=rec2[:], in0=pm2[:, 0:1], scalar1=EPS, scalar2=None,
                        op0=mybir.AluOpType.add)
                    nc.vector.reciprocal(out=rec2[:], in_=rec2[:])
                    ob = sm.tile([P, OUT], F32, tag="ob")
                    nc.vector.scalar_tensor_tensor(
                        out=ob[:], in0=pm2[:, 1:], scalar=rec2[:, 0:1], in1=b2_sb[:],
                        op0=mybir.AluOpType.mult, op1=mybir.AluOpType.add)
                    nrows = min(P, cfg.ND - b * P)
                    nc.scalar.dma_start(out=OUTT[b * P: b * P + nrows, :],
                                        in_=ob[0:nrows, :])
            _stack_c.close()
    return nc


# ---------------------------------------------------------------- entry point
def gat_run(cfg, x, edge_index, W1, att_src1, att_dst1, b1, W2, att_src2,
            att_dst2, b2, trace=False):
    x = np.asarray(x, dtype=np.float32)
    edge_index = np.asarray(edge_index)
    W1f, W2f = make_weights(cfg, np.asarray(W1, np.float64),
                            np.asarray(att_src1, np.float64),
                            np.asarray(att_dst1, np.float64),
                            np.asarray(W2, np.float64),
                            np.asarray(att_src2, np.float64),
                            np.asarray(att_dst2, np.float64))
    ei = edge_index.astype(np.int64)
    idx16, dlocf, dloct, TA, TB = preprocess_graph(cfg, ei)
    Tsum = sum(TA) + sum(TB)

    nc = build_kernel(cfg, TA, TB, Tsum)
    nc.finalize()

    P_, KI = P, cfg.IN_CH // P
    # node-order (core-major padded) X; XTI feeds phase-A compute by
    # (group, subblock) so it is independent of the table row layout
    xr = np.zeros((cfg.R, cfg.IN_CH), dtype=np.float32)
    for c in range(NCORES):
        xr[c * cfg.NPAD: c * cfg.NPAD + cfg.ND] = x[c * cfg.ND:(c + 1) * cfg.ND]
    # XTI[p, rb*KI*128 + k*128 + j] = xr[rb*128 + j, k*128 + p], with the
    # rb axis in slab-major group order (group gi = slab*8+core writes table1
    # rows [gi*896, (gi+1)*896) and reads blocks core*49 + slab*7 + s)
    xrT = np.ascontiguousarray(
        xr.reshape(cfg.RB, P_, KI, P_).transpose(3, 0, 2, 1))  # [p, rb, k, j]
    gborder = np.array(
        [c * cfg.NB + slab * cfg.GA + s
         for half in (range(cfg.SLO), range(cfg.SLO, cfg.NSLAB))
         for c in range(NCORES)
         for slab in half
         for s in range(cfg.GA)])
    xrT = np.ascontiguousarray(xrT[:, gborder])
    xti = bf16(xrT.reshape(P_, cfg.RB * KI * P_))

    w1f_r = bf16(W1f.reshape(KI, P_, cfg.U1 + cfg.H1).transpose(1, 0, 2))
    w2f_r = bf16(W2f.reshape(cfg.HID // P_, P_, cfg.U2 + 1).transpose(1, 0, 2))

    iota = bf16(np.broadcast_to(np.arange(P_, dtype=np.float32), (P_, P_)))
    iotac = np.arange(P_, dtype=np.float32)[:, None].copy()
    ones1 = bf16(np.ones((1, P_), dtype=np.float32))
    ident = np.eye(P_, dtype=np.float32)
    b1r = np.broadcast_to(np.asarray(b1, np.float32), (P_, cfg.HID)).copy()
    b2r = np.broadcast_to(np.asarray(b2, np.float32), (P_, cfg.OUT_CH)).copy()

    in_maps = []
    for c in range(NCORES):
        xsh = xr[c * cfg.NPAD:(c + 1) * cfg.NPAD]
        xshT = np.ascontiguousarray(
            xsh.reshape(cfg.NB, P_, KI, P_).transpose(3, 0, 2, 1))
        xsi = bf16(xshT.reshape(P_, cfg.NB * KI * P_))
        # merged tidx+dloc: per block segment, 8T idx cols then T dloc cols
        tdl = np.zeros((P_, 9 * Tsum), dtype=np.int16)
        dloc_i16 = bf16(dlocf[c]).view(np.int16)
        col = 0
        icol = 0
        for b in range(len(TA)):
            T = TA[b] + TB[b]
            tdl[:, col: col + 8 * T] = idx16[c][:, 8 * icol: 8 * (icol + T)]
            tdl[:, col + 8 * T: col + 9 * T] = dloc_i16[:, icol: icol + T]
            col += 9 * T
            icol += T
        in_maps.append({
            "XTI": xti, "XSI": xsi, "W1F": w1f_r, "W2F": w2f_r,
            "TDL": tdl, "DLOCT": bf16(dloct[c]),
            "IOTA": iota, "IOTAC": iotac, "ONES1": ones1, "IDENT": ident,
            "B1R": b1r, "B2R": b2r,
        })
    res = run_bass_kernel_spmd(nc, in_maps, list(range(NCORES)), trace=trace)
    out = np.concatenate([res.results[c]["OUTT"] for c in range(NCORES)], axis=0)
    return out[:cfg.N], res


def kernel(x, edge_index, W1, att_src1, att_dst1, b1, W2, att_src2, att_dst2,
           b2):
    out, _ = gat_run(CFG_FULL, x, edge_index, W1, att_src1, att_dst1, b1, W2,
                     att_src2, att_dst2, b2)
    return out.astype(np.float32)
